# revision 22
# baseline (speedup 1.0000x reference)
"""MoE (8 experts, top-2, + shared expert) for Trainium2, expert-parallel
across 8 NeuronCores.

Layout strategy:
  - Host computes the router (logits, top-2, softmax, aux losses) and
    dispatches: core e receives the tokens routed to expert e (gathered,
    padded to a uniform capacity) plus a 1/8 slice of all tokens for the
    shared expert.
  - Each core runs two fused FFNs (expert + shared):
      GEMM1:  hid^T[h, c] = gelu(sum_d w1[d, h] * x^T[d, c] + b1[h])
      GEMM2:  y[c, d]     = w_tok[c] * sum_h hid^T[h, c] * w2[h, d]
    Activations stay transposed between the GEMMs so no on-device
    transposes are needed; the router weight is a per-partition scalar.
  - Host scatter-adds the weighted expert outputs into the shared-expert
    output (the unshard step) and adds the (zero) output biases exactly.

Matmuls run in float32r (TF32-class, ~1.5e-4 rel err, 4x faster than fp32
on the PE array); set MOE_MM_DTYPE=float32 for exact-fp32 fallback.
"""

import os
import sys

import numpy as np

try:
    import concourse.bacc as bacc
except ImportError:  # pragma: no cover
    sys.path.insert(0, "/opt/trn_rl_repo")
    import concourse.bacc as bacc

import concourse.mybir as mybir
import concourse.tile as tile
from concourse.bass_utils import run_bass_kernel_spmd

B, S, D, H, E, TOPK = 4, 2048, 1024, 4096, 8, 2
N = B * S                     # 8192 tokens
N_CORES = 8
S_LOC = N // N_CORES          # shared-expert tokens per core
Z_COEF, BAL_COEF = 0.001, 0.01
KT = D // 128                 # 8 k-tiles over d_model
HT = H // 128                 # 32 h-tiles over hidden
CHUNK = 512                   # token columns per pipeline chunk

_F32 = mybir.dt.float32
_MM_DT = {
    "float32": mybir.dt.float32,
    "float32r": mybir.dt.float32r,
    "bfloat16": mybir.dt.bfloat16,
}[os.environ.get("MOE_MM_DTYPE", "float32r")]

_NC_CACHE: dict = {}

# set after each run when MOE_TRACE=1 (for the local test harness only)
LAST_EXEC_NS = None
LAST_RESULTS = None


def _build_nc(C: int):
    """Build the SPMD per-core program for expert capacity C (mult of 128)."""
    gelu = mybir.ActivationFunctionType.Gelu_apprx_tanh
    nc = bacc.Bacc("TRN2", target_bir_lowering=False)

    xe = nc.declare_dram_parameter("xe", [128, KT, C], _MM_DT, isOutput=False)
    w1 = nc.declare_dram_parameter("w1", [128, HT, KT, 128], _MM_DT, isOutput=False)
    w2 = nc.declare_dram_parameter("w2", [128, HT, D], _MM_DT, isOutput=False)
    b1 = nc.declare_dram_parameter("b1", [128, HT], _F32, isOutput=False)
    wt = nc.declare_dram_parameter("wt", [128, C // 128], _F32, isOutput=False)
    xs = nc.declare_dram_parameter("xs", [128, KT, S_LOC], _MM_DT, isOutput=False)
    ws1 = nc.declare_dram_parameter("ws1", [128, HT, KT, 128], _MM_DT, isOutput=False)
    ws2 = nc.declare_dram_parameter("ws2", [128, HT, D], _MM_DT, isOutput=False)
    bs1 = nc.declare_dram_parameter("bs1", [128, HT], _F32, isOutput=False)
    y = nc.declare_dram_parameter("y", [C, D], _F32, isOutput=True)
    ys = nc.declare_dram_parameter("ys", [S_LOC, D], _F32, isOutput=True)

    with tile.TileContext(nc) as tc:
        with (
            tc.tile_pool(name="const", bufs=1) as const,
            tc.tile_pool(name="xpool", bufs=1) as xpool,
            tc.tile_pool(name="hpool", bufs=1) as hpool,
            tc.tile_pool(name="w1pool", bufs=5) as w1pool,
            tc.tile_pool(name="w2pool", bufs=8) as w2pool,
            tc.tile_pool(name="evpool", bufs=4) as evpool,
            tc.tile_pool(name="psp", bufs=1, space="PSUM") as psp,
        ):
            # HAM warm-up: back-to-back matmuls on junk data so the PE
            # clock-gate reaches 8/8 while the first real DMAs are in flight
            warm = const.tile([128, 512], _F32, name="warm")
            nc.vector.memset(warm[:], 0.0)
            wps = psp.tile([128, 512], _F32, name="ps_0")
            for _ in range(64):
                nc.tensor.matmul(
                    wps[:, :128], warm[:, :128], warm[:, :128], start=True, stop=True
                )

            b1_sb = const.tile([128, HT], _F32, name="b1_sb")
            nc.sync.dma_start(out=b1_sb[:], in_=b1[:])
            bs1_sb = const.tile([128, HT], _F32, name="bs1_sb")
            nc.sync.dma_start(out=bs1_sb[:], in_=bs1[:])
            wt_sb = const.tile([128, C // 128], _F32, name="wt_sb")
            nc.sync.dma_start(out=wt_sb[:], in_=wt[:])

            # GEMM1 rotates starting at bank 6: banks 6,7 are never GEMM2
            # accumulators of a <=6-tile group, so the next group's GEMM1
            # can start while the previous group's GEMM2 still accumulates
            _g1_order = [6, 7, 0, 1, 2, 3, 4, 5]

            def psum_tile(tag):
                return psp.tile([128, 512], _F32, name=f"ps_{tag % 8}")

            def ffn(xd, w1d, w2d, b_sb, scale_sb, yd, ctot):
                # split the token dim into balanced groups of <=1024 columns
                # (each group = one pass over w1+w2); groups of <=768 leave
                # spare PSUM banks so the next group's GEMM1 overlaps GEMM2
                ngroups = -(-ctot // 1024)
                base = (ctot // ngroups) // 128 * 128
                widths = [base] * ngroups
                rem = (ctot - base * ngroups) // 128
                for i in range(rem):
                    widths[i] += 128
                pairs = []
                for gw in widths:
                    pairs.append(
                        tuple(w for w in (min(gw, CHUNK), gw - CHUNK) if w > 0)
                    )
                c0 = 0
                for pair in pairs:
                    offs = []
                    o = c0
                    for cw in pair:
                        offs.append(o)
                        o += cw
                    # load the pair's token columns
                    xts = []
                    for pi, cw in enumerate(pair):
                        xt = xpool.tile([128, KT, CHUNK], _MM_DT, name=f"xt{pi}")
                        for k in range(KT):
                            nc.gpsimd.dma_start(
                                out=xt[:, k, :cw],
                                in_=xd[:, k, offs[pi] : offs[pi] + cw],
                            )
                        xts.append(xt)
                    hids = [
                        hpool.tile([128, HT, CHUNK], _MM_DT, name=f"hid{pi}")
                        for pi in range(len(pair))
                    ]
                    # GEMM1 + gelu: one w1 panel load covers the whole pair
                    psi = 0
                    for h in range(HT):
                        w1t = w1pool.tile([128, KT, 128], _MM_DT, name="w1t")
                        nc.gpsimd.dma_start(out=w1t[:], in_=w1d[:, h, :, :])
                        for pi, cw in enumerate(pair):
                            ps = psum_tile(_g1_order[psi % 8])
                            psi += 1
                            for k in range(KT):
                                nc.tensor.matmul(
                                    ps[:, :cw],
                                    w1t[:, k, :],
                                    xts[pi][:, k, :cw],
                                    start=(k == 0),
                                    stop=(k == KT - 1),
                                )
                            nc.scalar.activation(
                                hids[pi][:, h, :cw],
                                ps[:, :cw],
                                gelu,
                                bias=b_sb[:, h : h + 1],
                            )
                    # GEMM2 (+ router-weight scale): accumulate all the pair's
                    # token tiles in PSUM while streaming w2 panels once
                    cps = []  # (hid, col0, global cp index)
                    for pi, cw in enumerate(pair):
                        for t in range(cw // 128):
                            cps.append((hids[pi], t * 128, offs[pi] // 128 + t))
                    for dh in range(2):
                        pss = [psum_tile(j) for j in range(len(cps))]
                        for h in range(HT):
                            w2t = w2pool.tile([128, 512], _MM_DT, name="w2t")
                            nc.sync.dma_start(
                                out=w2t[:], in_=w2d[:, h, dh * 512 : (dh + 1) * 512]
                            )
                            for j, (hidt, hc0, _) in enumerate(cps):
                                nc.tensor.matmul(
                                    pss[j][:],
                                    hidt[:, h, hc0 : hc0 + 128],
                                    w2t[:],
                                    start=(h == 0),
                                    stop=(h == HT - 1),
                                )
                        for j, (_, _, gcp) in enumerate(cps):
                            ev = evpool.tile([128, 512], _F32, name="ev")
                            # alternate DVE/ACT so the end-of-group evict
                            # burst drains on two engines in parallel
                            if scale_sb is None:
                                if j % 2 == 0:
                                    nc.vector.tensor_copy(ev[:], pss[j][:])
                                else:
                                    nc.scalar.copy(ev[:], pss[j][:])
                            else:
                                sc = scale_sb[:, gcp : gcp + 1]
                                if j % 2 == 0:
                                    nc.vector.tensor_scalar_mul(ev[:], pss[j][:], sc)
                                else:
                                    nc.scalar.activation(
                                        ev[:],
                                        pss[j][:],
                                        mybir.ActivationFunctionType.Copy,
                                        scale=sc,
                                    )
                            nc.sync.dma_start(
                                out=yd[
                                    gcp * 128 : (gcp + 1) * 128,
                                    dh * 512 : (dh + 1) * 512,
                                ],
                                in_=ev[:],
                            )
                    c0 = o

            ffn(xe, w1, w2, b1_sb, wt_sb, y, C)
            ffn(xs, ws1, ws2, bs1_sb, None, ys, S_LOC)

    nc.finalize()
    return nc


def _block4(w):  # [D, H] -> [128, HT, KT, 128]: [p, h, k, m] = w[k*128+p, h*128+m]
    return np.ascontiguousarray(
        w.reshape(KT, 128, HT, 128).transpose(1, 2, 0, 3)
    )


def _rows128(w):  # [H, D] -> [128, HT, D]: [p, h, d] = w[h*128+p, d]
    return np.ascontiguousarray(w.reshape(HT, 128, D).transpose(1, 0, 2))


def _cols_kmaj(xg, C):  # [cnt, D] -> [128, KT, C]: [p, k, c] = xg[c, k*128+p]
    out = np.zeros((128, KT, C), np.float32)
    out[:, :, : xg.shape[0]] = xg.T.reshape(KT, 128, -1).transpose(1, 0, 2)
    return out


def _pvec(v, n128):  # [n128*128] -> [128, n128]
    return np.ascontiguousarray(v.reshape(n128, 128).T)


def kernel(**inputs):
    global LAST_EXEC_NS, LAST_RESULTS
    x = np.asarray(inputs["x"], np.float32)
    gate_w = np.asarray(inputs["gate_w"], np.float32)
    keys_w = np.asarray(inputs["keys_w"], np.float32)
    keys_b = np.asarray(inputs["keys_b"], np.float32)
    values_w = np.asarray(inputs["values_w"], np.float32)
    values_b = np.asarray(inputs["values_b"], np.float32)
    shared_keys_w = np.asarray(inputs["shared_keys_w"], np.float32)
    shared_keys_b = np.asarray(inputs["shared_keys_b"], np.float32)
    shared_values_w = np.asarray(inputs["shared_values_w"], np.float32)
    shared_values_b = np.asarray(inputs["shared_values_b"], np.float32)

    xf = x.reshape(N, D)

    # ---- Router (host): top-2, softmax over selected, aux losses ----
    logits = xf @ gate_w                                  # [N, E] f32
    ar = np.arange(N)
    i0 = logits.argmax(axis=-1)
    masked = logits.copy()
    masked[ar, i0] = -np.inf
    i1 = masked.argmax(axis=-1)
    s0 = logits[ar, i0]
    s1 = logits[ar, i1]
    e1 = np.exp(s1 - s0)
    w0 = (1.0 / (1.0 + e1)).astype(np.float32)
    w1_ = (e1 / (1.0 + e1)).astype(np.float32)

    usage = (
        np.bincount(i0, weights=w0, minlength=E)
        + np.bincount(i1, weights=w1_, minlength=E)
    ).astype(np.float32)
    balance_loss = usage.std() / usage.mean() * BAL_COEF
    m = logits.max(axis=-1)
    lse = m + np.log(np.exp(logits - m[:, None]).sum(axis=-1))
    z_loss = np.square(lse).mean() * Z_COEF
    router_loss = np.float32(balance_loss + z_loss)

    # ---- Dispatch (host gather) ----
    toks, wtoks = [], []
    for e in range(E):
        sel0 = i0 == e
        sel1 = i1 == e
        tok = np.nonzero(sel0 | sel1)[0]
        wt = np.where(sel0[tok], w0[tok], w1_[tok]).astype(np.float32)
        toks.append(tok)
        wtoks.append(wt)
    cap = max(len(t) for t in toks)
    C = max(512, -(-cap // 128) * 128)

    ws1_dev = _block4(shared_keys_w)
    ws2_dev = _rows128(shared_values_w)
    bs1_dev = _pvec(shared_keys_b, HT)

    in_maps = []
    for e in range(E):
        wt_full = np.zeros(C, np.float32)
        wt_full[: len(toks[e])] = wtoks[e]
        in_maps.append(
            {
                "xe": _cols_kmaj(xf[toks[e]], C),
                "w1": _block4(keys_w[e]),
                "w2": _rows128(values_w[e]),
                "b1": _pvec(keys_b[e], HT),
                "wt": _pvec(wt_full, C // 128),
                "xs": _cols_kmaj(xf[e * S_LOC : (e + 1) * S_LOC], S_LOC),
                "ws1": ws1_dev,
                "ws2": ws2_dev,
                "bs1": bs1_dev,
            }
        )

    # ---- Device: compile (cached per capacity) + run ----
    key = (C, _MM_DT)
    if key not in _NC_CACHE:
        _NC_CACHE[key] = _build_nc(C)
    nc = _NC_CACHE[key]

    trace = bool(os.environ.get("MOE_TRACE"))
    res = run_bass_kernel_spmd(
        nc, in_maps, core_ids=list(range(N_CORES)), trace=trace
    )
    LAST_EXEC_NS = res.exec_time_ns
    LAST_RESULTS = res

    # ---- Combine (host scatter-add / unshard) ----
    out = np.empty((N, D), np.float32)
    for c in range(N_CORES):
        out[c * S_LOC : (c + 1) * S_LOC] = res.results[c]["ys"]
    out += shared_values_b[None, :]
    for e in range(E):
        cnt = len(toks[e])
        out[toks[e]] += (
            res.results[e]["y"][:cnt] + wtoks[e][:, None] * values_b[e][None, :]
        )

    return out.reshape(B, S, D), router_loss


# revision 23
# speedup vs baseline: 1.0074x; 1.0074x over previous
"""MoE (8 experts, top-2, + shared expert) for Trainium2, expert-parallel
across 8 NeuronCores.

Layout strategy:
  - Host computes the router (logits, top-2, softmax, aux losses) and
    dispatches: core e receives the tokens routed to expert e (gathered,
    padded to a uniform capacity) plus a 1/8 slice of all tokens for the
    shared expert.
  - Each core runs two fused FFNs (expert + shared):
      GEMM1:  hid^T[h, c] = gelu(sum_d w1[d, h] * x^T[d, c] + b1[h])
      GEMM2:  y[c, d]     = w_tok[c] * sum_h hid^T[h, c] * w2[h, d]
    Activations stay transposed between the GEMMs so no on-device
    transposes are needed; the router weight is a per-partition scalar.
  - Host scatter-adds the weighted expert outputs into the shared-expert
    output (the unshard step) and adds the (zero) output biases exactly.

Matmuls run in float32r (TF32-class, ~1.5e-4 rel err, 4x faster than fp32
on the PE array); set MOE_MM_DTYPE=float32 for exact-fp32 fallback.
"""

import os
import sys

import numpy as np

try:
    import concourse.bacc as bacc
except ImportError:  # pragma: no cover
    sys.path.insert(0, "/opt/trn_rl_repo")
    import concourse.bacc as bacc

import concourse.mybir as mybir
import concourse.tile as tile
from concourse.bass_utils import run_bass_kernel_spmd

B, S, D, H, E, TOPK = 4, 2048, 1024, 4096, 8, 2
N = B * S                     # 8192 tokens
N_CORES = 8
S_LOC = N // N_CORES          # shared-expert tokens per core
Z_COEF, BAL_COEF = 0.001, 0.01
KT = D // 128                 # 8 k-tiles over d_model
HT = H // 128                 # 32 h-tiles over hidden
CHUNK = 512                   # token columns per pipeline chunk

_F32 = mybir.dt.float32
_MM_DT = {
    "float32": mybir.dt.float32,
    "float32r": mybir.dt.float32r,
    "bfloat16": mybir.dt.bfloat16,
}[os.environ.get("MOE_MM_DTYPE", "float32r")]

_NC_CACHE: dict = {}

# set after each run when MOE_TRACE=1 (for the local test harness only)
LAST_EXEC_NS = None
LAST_RESULTS = None


def _build_nc(C: int):
    """Build the SPMD per-core program for expert capacity C (mult of 128)."""
    gelu = mybir.ActivationFunctionType.Gelu_apprx_tanh
    nc = bacc.Bacc("TRN2", target_bir_lowering=False)

    xe = nc.declare_dram_parameter("xe", [128, KT, C], _MM_DT, isOutput=False)
    w1 = nc.declare_dram_parameter("w1", [128, HT, KT, 128], _MM_DT, isOutput=False)
    w2 = nc.declare_dram_parameter("w2", [128, HT, D], _MM_DT, isOutput=False)
    b1 = nc.declare_dram_parameter("b1", [128, HT], _F32, isOutput=False)
    wt = nc.declare_dram_parameter("wt", [128, C // 128], _F32, isOutput=False)
    xs = nc.declare_dram_parameter("xs", [128, KT, S_LOC], _MM_DT, isOutput=False)
    ws1 = nc.declare_dram_parameter("ws1", [128, HT, KT, 128], _MM_DT, isOutput=False)
    ws2 = nc.declare_dram_parameter("ws2", [128, HT, D], _MM_DT, isOutput=False)
    bs1 = nc.declare_dram_parameter("bs1", [128, HT], _F32, isOutput=False)
    y = nc.declare_dram_parameter("y", [C, D], _F32, isOutput=True)
    ys = nc.declare_dram_parameter("ys", [S_LOC, D], _F32, isOutput=True)

    with tile.TileContext(nc) as tc:
        with (
            tc.tile_pool(name="const", bufs=1) as const,
            tc.tile_pool(name="xpool", bufs=1) as xpool,
            tc.tile_pool(name="hpool", bufs=1) as hpool,
            tc.tile_pool(name="w1pool", bufs=5) as w1pool,
            tc.tile_pool(name="w2pool", bufs=8) as w2pool,
            tc.tile_pool(name="evpool", bufs=4) as evpool,
            tc.tile_pool(name="psp", bufs=1, space="PSUM") as psp,
        ):
            # HAM warm-up: back-to-back matmuls on junk data so the PE
            # clock-gate reaches 8/8 while the first real DMAs are in flight
            warm = const.tile([128, 512], _F32, name="warm")
            nc.vector.memset(warm[:], 0.0)
            wps = psp.tile([128, 512], _F32, name="ps_0")
            for _ in range(40):
                nc.tensor.matmul(
                    wps[:, :128], warm[:, :128], warm[:, :128], start=True, stop=True
                )

            b1_sb = const.tile([128, HT], _F32, name="b1_sb")
            nc.sync.dma_start(out=b1_sb[:], in_=b1[:])
            bs1_sb = const.tile([128, HT], _F32, name="bs1_sb")
            nc.sync.dma_start(out=bs1_sb[:], in_=bs1[:])
            wt_sb = const.tile([128, C // 128], _F32, name="wt_sb")
            nc.sync.dma_start(out=wt_sb[:], in_=wt[:])

            # GEMM1 rotates starting at bank 6: banks 6,7 are never GEMM2
            # accumulators of a <=6-tile group, so the next group's GEMM1
            # can start while the previous group's GEMM2 still accumulates
            _g1_order = [6, 7, 0, 1, 2, 3, 4, 5]

            def psum_tile(tag):
                return psp.tile([128, 512], _F32, name=f"ps_{tag % 8}")

            def ffn(xd, w1d, w2d, b_sb, scale_sb, yd, ctot, first=False):
                # split the token dim into balanced groups of <=1024 columns
                # (each group = one pass over w1+w2); groups of <=768 leave
                # spare PSUM banks so the next group's GEMM1 overlaps GEMM2
                ngroups = -(-ctot // 1024)
                base = (ctot // ngroups) // 128 * 128
                widths = [base] * ngroups
                rem = (ctot - base * ngroups) // 128
                for i in range(rem):
                    widths[i] += 128
                pairs = []
                for gw in widths:
                    pairs.append(
                        tuple(w for w in (min(gw, CHUNK), gw - CHUNK) if w > 0)
                    )
                c0 = 0
                for gi, pair in enumerate(pairs):
                    offs = []
                    o = c0
                    for cw in pair:
                        offs.append(o)
                        o += cw
                    # load the pair's token columns
                    xts = []
                    # kernel start: the w2 stream hasn't begun, so the Sync
                    # queue is idle — prime the first pair's tokens there in
                    # parallel with the w1 panels on GpSimd
                    xq = nc.sync if (first and gi == 0) else nc.gpsimd
                    for pi, cw in enumerate(pair):
                        xt = xpool.tile([128, KT, CHUNK], _MM_DT, name=f"xt{pi}")
                        for k in range(KT):
                            xq.dma_start(
                                out=xt[:, k, :cw],
                                in_=xd[:, k, offs[pi] : offs[pi] + cw],
                            )
                        xts.append(xt)
                    hids = [
                        hpool.tile([128, HT, CHUNK], _MM_DT, name=f"hid{pi}")
                        for pi in range(len(pair))
                    ]
                    # GEMM1 + gelu: one w1 panel load covers the whole pair
                    psi = 0
                    for h in range(HT):
                        w1t = w1pool.tile([128, KT, 128], _MM_DT, name="w1t")
                        nc.gpsimd.dma_start(out=w1t[:], in_=w1d[:, h, :, :])
                        for pi, cw in enumerate(pair):
                            ps = psum_tile(_g1_order[psi % 8])
                            psi += 1
                            for k in range(KT):
                                nc.tensor.matmul(
                                    ps[:, :cw],
                                    w1t[:, k, :],
                                    xts[pi][:, k, :cw],
                                    start=(k == 0),
                                    stop=(k == KT - 1),
                                )
                            nc.scalar.activation(
                                hids[pi][:, h, :cw],
                                ps[:, :cw],
                                gelu,
                                bias=b_sb[:, h : h + 1],
                            )
                    # GEMM2 (+ router-weight scale): accumulate all the pair's
                    # token tiles in PSUM while streaming w2 panels once
                    cps = []  # (hid, col0, global cp index)
                    for pi, cw in enumerate(pair):
                        for t in range(cw // 128):
                            cps.append((hids[pi], t * 128, offs[pi] // 128 + t))
                    for dh in range(2):
                        pss = [psum_tile(j) for j in range(len(cps))]
                        for h in range(HT):
                            w2t = w2pool.tile([128, 512], _MM_DT, name="w2t")
                            nc.sync.dma_start(
                                out=w2t[:], in_=w2d[:, h, dh * 512 : (dh + 1) * 512]
                            )
                            for j, (hidt, hc0, _) in enumerate(cps):
                                nc.tensor.matmul(
                                    pss[j][:],
                                    hidt[:, h, hc0 : hc0 + 128],
                                    w2t[:],
                                    start=(h == 0),
                                    stop=(h == HT - 1),
                                )
                        for j, (_, _, gcp) in enumerate(cps):
                            ev = evpool.tile([128, 512], _F32, name="ev")
                            # alternate DVE/ACT so the end-of-group evict
                            # burst drains on two engines in parallel
                            if scale_sb is None:
                                if j % 2 == 0:
                                    nc.vector.tensor_copy(ev[:], pss[j][:])
                                else:
                                    nc.scalar.copy(ev[:], pss[j][:])
                            else:
                                sc = scale_sb[:, gcp : gcp + 1]
                                if j % 2 == 0:
                                    nc.vector.tensor_scalar_mul(ev[:], pss[j][:], sc)
                                else:
                                    nc.scalar.activation(
                                        ev[:],
                                        pss[j][:],
                                        mybir.ActivationFunctionType.Copy,
                                        scale=sc,
                                    )
                            nc.sync.dma_start(
                                out=yd[
                                    gcp * 128 : (gcp + 1) * 128,
                                    dh * 512 : (dh + 1) * 512,
                                ],
                                in_=ev[:],
                            )
                    c0 = o

            ffn(xe, w1, w2, b1_sb, wt_sb, y, C, first=True)
            ffn(xs, ws1, ws2, bs1_sb, None, ys, S_LOC)

    nc.finalize()
    return nc


def _block4(w):  # [D, H] -> [128, HT, KT, 128]: [p, h, k, m] = w[k*128+p, h*128+m]
    return np.ascontiguousarray(
        w.reshape(KT, 128, HT, 128).transpose(1, 2, 0, 3)
    )


def _rows128(w):  # [H, D] -> [128, HT, D]: [p, h, d] = w[h*128+p, d]
    return np.ascontiguousarray(w.reshape(HT, 128, D).transpose(1, 0, 2))


def _cols_kmaj(xg, C):  # [cnt, D] -> [128, KT, C]: [p, k, c] = xg[c, k*128+p]
    out = np.zeros((128, KT, C), np.float32)
    out[:, :, : xg.shape[0]] = xg.T.reshape(KT, 128, -1).transpose(1, 0, 2)
    return out


def _pvec(v, n128):  # [n128*128] -> [128, n128]
    return np.ascontiguousarray(v.reshape(n128, 128).T)


def kernel(**inputs):
    global LAST_EXEC_NS, LAST_RESULTS
    x = np.asarray(inputs["x"], np.float32)
    gate_w = np.asarray(inputs["gate_w"], np.float32)
    keys_w = np.asarray(inputs["keys_w"], np.float32)
    keys_b = np.asarray(inputs["keys_b"], np.float32)
    values_w = np.asarray(inputs["values_w"], np.float32)
    values_b = np.asarray(inputs["values_b"], np.float32)
    shared_keys_w = np.asarray(inputs["shared_keys_w"], np.float32)
    shared_keys_b = np.asarray(inputs["shared_keys_b"], np.float32)
    shared_values_w = np.asarray(inputs["shared_values_w"], np.float32)
    shared_values_b = np.asarray(inputs["shared_values_b"], np.float32)

    xf = x.reshape(N, D)

    # ---- Router (host): top-2, softmax over selected, aux losses ----
    logits = xf @ gate_w                                  # [N, E] f32
    ar = np.arange(N)
    i0 = logits.argmax(axis=-1)
    masked = logits.copy()
    masked[ar, i0] = -np.inf
    i1 = masked.argmax(axis=-1)
    s0 = logits[ar, i0]
    s1 = logits[ar, i1]
    e1 = np.exp(s1 - s0)
    w0 = (1.0 / (1.0 + e1)).astype(np.float32)
    w1_ = (e1 / (1.0 + e1)).astype(np.float32)

    usage = (
        np.bincount(i0, weights=w0, minlength=E)
        + np.bincount(i1, weights=w1_, minlength=E)
    ).astype(np.float32)
    balance_loss = usage.std() / usage.mean() * BAL_COEF
    m = logits.max(axis=-1)
    lse = m + np.log(np.exp(logits - m[:, None]).sum(axis=-1))
    z_loss = np.square(lse).mean() * Z_COEF
    router_loss = np.float32(balance_loss + z_loss)

    # ---- Dispatch (host gather) ----
    toks, wtoks = [], []
    for e in range(E):
        sel0 = i0 == e
        sel1 = i1 == e
        tok = np.nonzero(sel0 | sel1)[0]
        wt = np.where(sel0[tok], w0[tok], w1_[tok]).astype(np.float32)
        toks.append(tok)
        wtoks.append(wt)
    cap = max(len(t) for t in toks)
    C = max(512, -(-cap // 128) * 128)

    ws1_dev = _block4(shared_keys_w)
    ws2_dev = _rows128(shared_values_w)
    bs1_dev = _pvec(shared_keys_b, HT)

    in_maps = []
    for e in range(E):
        wt_full = np.zeros(C, np.float32)
        wt_full[: len(toks[e])] = wtoks[e]
        in_maps.append(
            {
                "xe": _cols_kmaj(xf[toks[e]], C),
                "w1": _block4(keys_w[e]),
                "w2": _rows128(values_w[e]),
                "b1": _pvec(keys_b[e], HT),
                "wt": _pvec(wt_full, C // 128),
                "xs": _cols_kmaj(xf[e * S_LOC : (e + 1) * S_LOC], S_LOC),
                "ws1": ws1_dev,
                "ws2": ws2_dev,
                "bs1": bs1_dev,
            }
        )

    # ---- Device: compile (cached per capacity) + run ----
    key = (C, _MM_DT)
    if key not in _NC_CACHE:
        _NC_CACHE[key] = _build_nc(C)
    nc = _NC_CACHE[key]

    trace = bool(os.environ.get("MOE_TRACE"))
    res = run_bass_kernel_spmd(
        nc, in_maps, core_ids=list(range(N_CORES)), trace=trace
    )
    LAST_EXEC_NS = res.exec_time_ns
    LAST_RESULTS = res

    # ---- Combine (host scatter-add / unshard) ----
    out = np.empty((N, D), np.float32)
    for c in range(N_CORES):
        out[c * S_LOC : (c + 1) * S_LOC] = res.results[c]["ys"]
    out += shared_values_b[None, :]
    for e in range(E):
        cnt = len(toks[e])
        out[toks[e]] += (
            res.results[e]["y"][:cnt] + wtoks[e][:, None] * values_b[e][None, :]
        )

    return out.reshape(B, S, D), router_loss
